# revision 21
# baseline (speedup 1.0000x reference)
"""Multi-head dot-product attention (B=2, T=2048, D=1024, H=16, HD=64)
on 8 NeuronCores.

Sharding: core = b*4 + g  (b = batch index, g = head-group of 4 heads).
Each core computes the Q/K/V projections for its 256 W-columns, causal
attention for its 4 heads of one batch, and a partial output projection
through its 256 Wo rows.  The host sums the 4 partials per batch (this
is the Wo row-split all-reduce) and stacks the 2 batches.

Per-core kernel layout notes:
  - x (inputs) are PE-transposed on chip into xT[d, t] so both the
    projection matmuls (contraction over D) and the attention matmuls
    (contraction over HD / T') see their contraction dim on partitions.
  - Attention computes S^T[T', t] = K Q^T per (head, 128-row T'-tile,
    512-col t-block), exponentiates on ScalarE (scale=1/8 fused, no max
    subtraction: scores are ~N(0,1) so exp never overflows in fp32),
    applies the causal mask multiplicatively for diagonal blocks, and
    feeds A^T straight into O^T = V_aug^T A^T where V_aug carries a
    ones column that yields the softmax denominator for free.
  - All matmuls run in float32r (full-rate fp32 mode on TRN2 PE).
"""

import os
import sys
from contextlib import ExitStack

import numpy as np

if "/opt/trn_rl_repo" not in sys.path:
    sys.path.insert(0, "/opt/trn_rl_repo")

import concourse.bass as bass  # noqa: E402
import concourse.mybir as mybir  # noqa: E402
import concourse.tile as tile  # noqa: E402
from concourse import bacc  # noqa: E402
from concourse.bass import ts  # noqa: E402
from concourse.bass_utils import run_bass_kernel_spmd  # noqa: E402


P = 128
D = 1024
GD = 256  # W-columns / Wo-rows per core
NH = 4  # heads per core
HD = 64
DC = D // P  # 8 contraction chunks over D
N_CORES = 8
B = 2

F32 = mybir.dt.float32
F32R = mybir.dt.float32r
F16 = mybir.dt.float16
EXP = mybir.ActivationFunctionType.Exp

_cache: dict = {}
LAST_RESULTS = None


def _enable_tracing():
    """Register the axon NTFF profiling hook (missing antenv.axon_hooks shim)
    and neuter the artifact upload so trace=True works in this container."""
    import types

    try:
        import antenv.axon_hooks  # noqa: F401
    except ImportError:
        import antenv

        mod = types.ModuleType("antenv.axon_hooks")
        _hook = [None]
        mod.set_axon_ntff_profile_hook = lambda h: _hook.__setitem__(0, h)
        mod.get_axon_ntff_profile_hook = lambda: _hook[0]
        sys.modules["antenv.axon_hooks"] = mod
        antenv.axon_hooks = mod
        try:
            if "/root/.axon_site" not in sys.path:
                sys.path.append("/root/.axon_site")
            from trn_agent_boot.trn_boot import _ntff_profile_via_ctypes

            so = "/opt/axon/libaxon_pjrt.so"
            if os.path.exists(so):
                hook = _ntff_profile_via_ctypes(so)
                if hook is not None:
                    mod.set_axon_ntff_profile_hook(hook)
        except Exception:
            pass
    import concourse.bass_utils as _bu

    _bu.upload_artifacts = lambda tmpdir: tmpdir


def _classify(mask2d: np.ndarray, T: int, BN: int):
    """Classify each (tau, j) attention block of the shared [T, T] mask.

    Returns (mask_kind, pattern, n_mixed, mix_arr) where pattern[tau][j] is
    "full" / "skip" / ("strip", o) / ("mix", idx).  The kernel sees S^T
    blocks: rows T' in [128*tau, 128*tau+128), cols t in [BN*j, BN*j+BN),
    valid iff mask[t, T'].
    """
    NT, NJ = T // P, T // BN
    if mask2d.all():
        pattern = tuple(tuple("full" for _ in range(NJ)) for _ in range(NT))
        return "full", pattern, 0, None
    tril = np.tril(np.ones((T, T), dtype=bool))
    if np.array_equal(mask2d, tril):
        pat = []
        for tau in range(NT):
            row = []
            for j in range(NJ):
                if P * tau + P - 1 <= BN * j:
                    row.append("full")
                elif P * tau > BN * j + BN - 1:
                    row.append("skip")
                else:
                    row.append(("strip", P * tau - BN * j))
            pat.append(tuple(row))
        return "tril", tuple(pat), 0, None
    # general mask: ship the mixed blocks as multiplicative tiles
    maskT = mask2d.T
    pat = []
    mixes = []
    for tau in range(NT):
        row = []
        for j in range(NJ):
            sub = maskT[tau * P : (tau + 1) * P, j * BN : (j + 1) * BN]
            if sub.all():
                row.append("full")
            elif not sub.any():
                row.append("skip")
            else:
                row.append(("mix", len(mixes)))
                mixes.append(sub.astype(np.float32))
        pat.append(tuple(row))
    mix_arr = (
        np.stack(mixes, axis=1) if mixes else np.zeros((P, 1, BN), np.float32)
    )
    return "general", tuple(pat), len(mixes), np.ascontiguousarray(mix_arr)


def _build(T: int, BN: int, pattern, n_mixed: int, mask_kind: str):
    NT = T // P
    NJ = T // BN
    W = 2 * BN - P  # causal strip width
    MAXO = BN - P
    PJ = min(512, T)  # matmul free-dim chunk for projections
    NPJ = T // PJ

    nc = bacc.Bacc("TRN2", target_bir_lowering=False, debug=False)
    xq = nc.dram_tensor("xq", [T, D], F32, kind="ExternalInput").ap()
    xkv = nc.dram_tensor("xkv", [T, D], F32, kind="ExternalInput").ap()
    wq = nc.dram_tensor("wq", [D, GD], F32, kind="ExternalInput").ap()
    wk = nc.dram_tensor("wk", [D, GD], F32, kind="ExternalInput").ap()
    wv = nc.dram_tensor("wv", [D, GD], F32, kind="ExternalInput").ap()
    wo = nc.dram_tensor("wo", [GD, D], F32, kind="ExternalInput").ap()
    ident_in = nc.dram_tensor("ident", [P, P], F16, kind="ExternalInput").ap()
    ones_in = nc.dram_tensor("ones", [P, NT, 2], F16, kind="ExternalInput").ap()
    hi_in = nc.dram_tensor("hi_init", [P, NT, 2, HD], F16, kind="ExternalInput").ap()
    sel_in = nc.dram_tensor("sel", [NH, 2, P], F16, kind="ExternalInput").ap()
    mstrip = mmix = None
    if mask_kind == "tril":
        mstrip = nc.dram_tensor("mstrip", [P, W], F16, kind="ExternalInput").ap()
    elif mask_kind == "general":
        mmix = nc.dram_tensor(
            "mmix", [P, max(n_mixed, 1), BN], F16, kind="ExternalInput"
        ).ap()
    out = nc.dram_tensor("out", [T, D], F32, kind="ExternalOutput").ap()

    with tile.TileContext(nc) as tc, ExitStack() as ctx:
        pp = ctx.enter_context(tc.tile_pool(name="persist", bufs=1))
        xt = ctx.enter_context(tc.tile_pool(name="xt", bufs=DC + 2))
        p_xrow = ctx.enter_context(tc.tile_pool(name="xrow", bufs=3))
        p_ws = ctx.enter_context(tc.tile_pool(name="wstage", bufs=2))
        p_at = ctx.enter_context(tc.tile_pool(name="atp", bufs=4))
        p_sm = ctx.enter_context(tc.tile_pool(name="small", bufs=2))
        ps_t = ctx.enter_context(tc.tile_pool(name="ps_t", bufs=1, space="PSUM"))
        ps_mm = ctx.enter_context(tc.tile_pool(name="ps_mm", bufs=3, space="PSUM"))
        ps_v = ctx.enter_context(tc.tile_pool(name="ps_v", bufs=2, space="PSUM"))
        ps_o = ctx.enter_context(tc.tile_pool(name="ps_o", bufs=2, space="PSUM"))

        ident = pp.tile([P, P], F16, tag="ident")
        nc.sync.dma_start(ident, ident_in)

        msk = None
        if mask_kind == "tril":
            msk = pp.tile([P, W], F16, tag="mstrip")
            nc.sync.dma_start(msk, mstrip)
        elif mask_kind == "general":
            msk = pp.tile([P, max(n_mixed, 1), BN], F16, tag="mmix")
            nc.sync.dma_start(msk, mmix)

        def load_w16(src_ap, shape, tag):
            stage = p_ws.tile(shape, F32, tag="wstage")
            nc.sync.dma_start(stage, src_ap)
            w16 = pp.tile(shape, F16, tag=tag)
            nc.vector.tensor_copy(w16, stage)
            return w16

        wq_sb = load_w16(wq.rearrange("(c p) g -> p c g", p=P), [P, DC, GD], "wq")
        wk_sb = load_w16(wk.rearrange("(c p) g -> p c g", p=P), [P, DC, GD], "wk")
        wv_sb = load_w16(wv.rearrange("(c p) g -> p c g", p=P), [P, DC, GD], "wv")
        wo_sb = load_w16(
            wo.rearrange("(c p) n -> p c n", p=P), [P, GD // P, D], "wo"
        )

        KT = pp.tile([P, 2, T], F16, tag="KT")  # [d%128, d//128, T']
        QT = pp.tile([P, 2, T], F16, tag="QT")
        # Even heads (OgT partitions 0-63): V in cols 0-63, ones col at 64
        #   -> out partitions 0-63 = O^T, partition 64 = denominator.
        # Odd heads (OgT partitions 64-127): ones col 0, zeros 1-63, V in
        #   cols 64-127 -> partition 0 = denominator (partition_broadcast
        #   only reads absolute partition 0 on HW), partitions 64-127 = O^T.
        Vt_lo = pp.tile([P, NT, 2, HD + 1], F16, tag="Vt_lo")
        Vt_hi = pp.tile([P, NT, 2, P], F16, tag="Vt_hi")
        OgT = pp.tile([P, GD // P, T], F16, tag="OgT")
        nc.sync.dma_start(Vt_lo[:, :, :, HD], ones_in)
        nc.sync.dma_start(Vt_hi[:, :, :, 0:HD], hi_in)
        sel_sb = pp.tile([NH, 2, P], F16, tag="sel")
        nc.sync.dma_start(sel_sb, sel_in)

        def transpose_in(src, dst_tiles):
            for tt in range(NT):
                xrow = p_xrow.tile([P, D], F32, tag="xrow")
                nc.sync.dma_start(xrow, src[ts(tt, P), :])
                xcast = p_xrow.tile([P, D], F16, tag="xcast")
                nc.vector.tensor_copy(xcast, xrow)
                for c in range(DC):
                    pt = ps_t.tile([P, P], F16, tag="tp")
                    nc.tensor.transpose(pt, xcast[:, ts(c, P)], ident)
                    nc.any.tensor_copy(dst_tiles[c][:, ts(tt, P)], pt)

        def project(xT_tiles, w_sb, dstT):
            # dstT[d, t] = sum_D w[D, d] * x[t, D], d-major output
            for dc in range(GD // P):
                for jn in range(NPJ):
                    ps = ps_mm.tile([P, PJ], F32, tag="mm")
                    for c in range(DC):
                        nc.tensor.matmul(
                            ps,
                            lhsT=w_sb[:, c, ts(dc, P)],
                            rhs=xT_tiles[c][:, ts(jn, PJ)],
                            start=(c == 0),
                            stop=(c == DC - 1),
                        )
                    nc.any.tensor_copy(dstT[:, dc, ts(jn, PJ)], ps)

        # ---- K, V from transposed xkv ----
        xkvT = [
            xt.tile([P, T], F16, tag="xt", name=f"xkvT{c}") for c in range(DC)
        ]
        transpose_in(xkv, xkvT)
        project(xkvT, wk_sb, KT)
        for tt in range(NT):
            ps = ps_v.tile([P, GD], F32, tag="vp")
            for c in range(DC):
                nc.tensor.matmul(
                    ps,
                    lhsT=xkvT[c][:, ts(tt, P)],
                    rhs=wv_sb[:, c, :],
                    start=(c == 0),
                    stop=(c == DC - 1),
                )
            for h in range(NH):
                if h % 2 == 0:
                    nc.any.tensor_copy(
                        Vt_lo[:, tt, h // 2, 0:HD], ps[:, ts(h, HD)]
                    )
                else:
                    nc.any.tensor_copy(
                        Vt_hi[:, tt, h // 2, HD:P], ps[:, ts(h, HD)]
                    )

        # ---- Q from transposed xq (reuses the xt slots) ----
        xqT = [xt.tile([P, T], F16, tag="xt", name=f"xqT{c}") for c in range(DC)]
        transpose_in(xq, xqT)
        project(xqT, wq_sb, QT)

        # ---- attention ----
        for j in range(NJ):
            den4 = p_sm.tile([NH, BN], F32, tag="den4")
            for h in range(NH):
                dc, dp = h // 2, (h % 2) * HD
                lo = h % 2 == 0
                po = ps_o.tile([P, BN], F32, tag="po")
                taus = [tau for tau in range(NT) if pattern[tau][j] != "skip"]
                for i, tau in enumerate(taus):
                    blk = pattern[tau][j]
                    # cols [0, off) of this S^T block are fully masked for
                    # every partition row: skip them in matmul/exp/mask.
                    off = blk[1] if blk != "full" and blk[0] == "strip" else 0
                    bw = BN - off
                    ps = ps_mm.tile([P, BN], F32, tag="mm")
                    nc.tensor.matmul(
                        ps[:, off:BN],
                        lhsT=KT[dp : dp + HD, dc, ts(tau, P)],
                        rhs=QT[dp : dp + HD, dc, j * BN + off : (j + 1) * BN],
                        start=True,
                        stop=True,
                    )
                    at = p_at.tile([P, BN], F16, tag="at")
                    nc.scalar.activation(
                        at[:, off:BN], ps[:, off:BN], EXP, scale=0.125
                    )
                    if blk != "full":
                        if blk[0] == "strip":
                            m_ap = msk[:, MAXO : MAXO + bw]
                        else:
                            m_ap = msk[:, blk[1], off:BN]
                        nc.vector.tensor_mul(
                            out=at[:, off:BN], in0=at[:, off:BN], in1=m_ap
                        )
                    vt = (
                        Vt_lo[:, tau, h // 2, :] if lo else Vt_hi[:, tau, h // 2, :]
                    )
                    nc.tensor.matmul(
                        po[0 : HD + 1, off:BN] if lo else po[:, off:BN],
                        lhsT=vt,
                        rhs=at[:, off:BN],
                        start=(i == 0),
                        stop=(i == len(taus) - 1),
                    )
                # stash the denominator row (psum -> sbuf, partition-aligned),
                # hop it into den4[h] (cross-partition via DMA), and copy the
                # unnormalized O^T out so the psum tile frees early.
                dp0 = HD if lo else 0
                stg = p_sm.tile([HD + 1, BN], F32, tag="dstage")
                nc.any.tensor_copy(stg[dp0 : dp0 + 1, :], po[dp0 : dp0 + 1, :])
                nc.sync.dma_start(den4[h : h + 1, :], stg[dp0 : dp0 + 1, :])
                nc.any.tensor_copy(
                    OgT[dp : dp + HD, dc, ts(j, BN)],
                    po[0:HD, :] if lo else po[HD:P, :],
                )
            # one batched reciprocal for all 4 heads of this t-block
            rec4 = p_sm.tile([NH, BN], F16, tag="rec4")
            with nc.allow_low_precision(reason="softmax denom recip in fp16"):
                nc.vector.reciprocal(rec4, den4)
            for c in range(2):
                # bc[p, f] = rec4[head(c, p), f] via a K=4 selection matmul
                bc = ps_mm.tile([P, BN], F32, tag="mm")
                nc.tensor.matmul(
                    bc,
                    lhsT=sel_sb[:, c, :],
                    rhs=rec4,
                    start=True,
                    stop=True,
                )
                nc.vector.tensor_mul(
                    out=OgT[:, c, ts(j, BN)],
                    in0=OgT[:, c, ts(j, BN)],
                    in1=bc,
                )

        # ---- output projection (partial: this core's 256 Wo rows) ----
        for tt in range(NT):
            for n in range(D // PJ):
                ps = ps_mm.tile([P, PJ], F32, tag="mm")
                for c in range(GD // P):
                    nc.tensor.matmul(
                        ps,
                        lhsT=OgT[:, c, ts(tt, P)],
                        rhs=wo_sb[:, c, ts(n, PJ)],
                        start=(c == 0),
                        stop=(c == GD // P - 1),
                    )
                ot = p_sm.tile([P, PJ], F32, tag="ot")
                nc.any.tensor_copy(ot, ps)
                nc.sync.dma_start(out[ts(tt, P), ts(n, PJ)], ot)

    nc.compile()
    return nc


def _strip(BN: int) -> np.ndarray:
    W, MAXO = 2 * BN - P, BN - P
    return np.ascontiguousarray(
        (np.arange(W)[None, :] >= (np.arange(P)[:, None] + MAXO)).astype(np.float32)
    )


def kernel(inputs_q, inputs_kv, mask, Wq, Wk, Wv, Wo):
    global LAST_RESULTS
    inputs_q = np.asarray(inputs_q, np.float32)
    inputs_kv = np.asarray(inputs_kv, np.float32)
    Wq, Wk, Wv, Wo = (np.asarray(w, np.float32) for w in (Wq, Wk, Wv, Wo))
    T = inputs_q.shape[1]
    BN = min(512, T)
    mask2d = np.asarray(mask).reshape(T, T).astype(bool)
    mask_kind, pattern, n_mixed, mix_arr = _classify(mask2d, T, BN)
    key = (T, BN, mask_kind, pattern)
    if key not in _cache:
        _cache[key] = _build(T, BN, pattern, n_mixed, mask_kind)
    nc = _cache[key]

    in_maps = []
    for core in range(N_CORES):
        b, g = divmod(core, N_CORES // B)
        m = {
            "xq": np.ascontiguousarray(inputs_q[b]),
            "xkv": np.ascontiguousarray(inputs_kv[b]),
            "wq": np.ascontiguousarray(Wq[:, g * GD : (g + 1) * GD]),
            "wk": np.ascontiguousarray(Wk[:, g * GD : (g + 1) * GD]),
            "wv": np.ascontiguousarray(Wv[:, g * GD : (g + 1) * GD]),
            "wo": np.ascontiguousarray(Wo[g * GD : (g + 1) * GD, :]),
        }
        m["ident"] = np.ascontiguousarray(np.eye(P, dtype=np.float16))
        sel = np.zeros((NH, 2, P), np.float16)
        for h in range(NH):
            sel[h, h // 2, (h % 2) * HD : (h % 2) * HD + HD] = 1.0
        m["sel"] = sel
        m["ones"] = np.ones((P, T // P, 2), np.float16)
        hi_init = np.zeros((P, T // P, 2, HD), np.float16)
        hi_init[..., 0] = 1.0
        m["hi_init"] = hi_init
        if mask_kind == "tril":
            m["mstrip"] = _strip(BN).astype(np.float16)
        elif mask_kind == "general":
            m["mmix"] = mix_arr.astype(np.float16)
        in_maps.append(m)

    trace = os.environ.get("KERNEL_TRACE", "0") == "1"
    if trace:
        _enable_tracing()
    res = run_bass_kernel_spmd(
        nc, in_maps, core_ids=list(range(N_CORES)), trace=trace
    )
    LAST_RESULTS = res
    parts = [res.results[c]["out"] for c in range(N_CORES)]
    gpb = N_CORES // B
    out = np.stack(
        [sum(parts[b * gpb : (b + 1) * gpb]) for b in range(B)], axis=0
    )
    return out.astype(np.float32)


# revision 22
# speedup vs baseline: 1.2025x; 1.2025x over previous
"""Multi-head dot-product attention (B=2, T=2048, D=1024, H=16, HD=64)
on 8 NeuronCores.

Sharding: core = b*4 + g  (b = batch index, g = head-group of 4 heads).
Each core computes the Q/K/V projections for its 256 W-columns, causal
attention for its 4 heads of one batch, and a partial output projection
through its 256 Wo rows.  The host sums the 4 partials per batch (this
is the Wo row-split all-reduce) and stacks the 2 batches.

Per-core kernel layout notes:
  - x (inputs) are PE-transposed on chip into xT[d, t] so both the
    projection matmuls (contraction over D) and the attention matmuls
    (contraction over HD / T') see their contraction dim on partitions.
  - Attention computes S^T[T', t] = K Q^T per (head, 128-row T'-tile,
    512-col t-block), exponentiates on ScalarE (scale=1/8 fused, no max
    subtraction: scores are ~N(0,1) so exp never overflows in fp32),
    applies the causal mask multiplicatively for diagonal blocks, and
    feeds A^T straight into O^T = V_aug^T A^T where V_aug carries a
    ones column that yields the softmax denominator for free.
  - All matmuls run in float32r (full-rate fp32 mode on TRN2 PE).
"""

import os
import sys
from contextlib import ExitStack

import numpy as np

if "/opt/trn_rl_repo" not in sys.path:
    sys.path.insert(0, "/opt/trn_rl_repo")

import concourse.bass as bass  # noqa: E402
import concourse.mybir as mybir  # noqa: E402
import concourse.tile as tile  # noqa: E402
from concourse import bacc  # noqa: E402
from concourse.bass import ts  # noqa: E402
from concourse.bass_utils import run_bass_kernel_spmd  # noqa: E402


P = 128
D = 1024
GD = 256  # W-columns / Wo-rows per core
NH = 4  # heads per core
HD = 64
DC = D // P  # 8 contraction chunks over D
N_CORES = 8
B = 2

F32 = mybir.dt.float32
F32R = mybir.dt.float32r
F16 = mybir.dt.float16
EXP = mybir.ActivationFunctionType.Exp

_cache: dict = {}
LAST_RESULTS = None


def _enable_tracing():
    """Register the axon NTFF profiling hook (missing antenv.axon_hooks shim)
    and neuter the artifact upload so trace=True works in this container."""
    import types

    try:
        import antenv.axon_hooks  # noqa: F401
    except ImportError:
        import antenv

        mod = types.ModuleType("antenv.axon_hooks")
        _hook = [None]
        mod.set_axon_ntff_profile_hook = lambda h: _hook.__setitem__(0, h)
        mod.get_axon_ntff_profile_hook = lambda: _hook[0]
        sys.modules["antenv.axon_hooks"] = mod
        antenv.axon_hooks = mod
        try:
            if "/root/.axon_site" not in sys.path:
                sys.path.append("/root/.axon_site")
            from trn_agent_boot.trn_boot import _ntff_profile_via_ctypes

            so = "/opt/axon/libaxon_pjrt.so"
            if os.path.exists(so):
                hook = _ntff_profile_via_ctypes(so)
                if hook is not None:
                    mod.set_axon_ntff_profile_hook(hook)
        except Exception:
            pass
    import concourse.bass_utils as _bu

    _bu.upload_artifacts = lambda tmpdir: tmpdir


def _classify(mask2d: np.ndarray, T: int, BN: int):
    """Classify each (tau, j) attention block of the shared [T, T] mask.

    Returns (mask_kind, pattern, n_mixed, mix_arr) where pattern[tau][j] is
    "full" / "skip" / ("strip", o) / ("mix", idx).  The kernel sees S^T
    blocks: rows T' in [128*tau, 128*tau+128), cols t in [BN*j, BN*j+BN),
    valid iff mask[t, T'].
    """
    NT, NJ = T // P, T // BN
    if mask2d.all():
        pattern = tuple(tuple("full" for _ in range(NJ)) for _ in range(NT))
        return "full", pattern, 0, None
    tril = np.tril(np.ones((T, T), dtype=bool))
    if np.array_equal(mask2d, tril):
        pat = []
        for tau in range(NT):
            row = []
            for j in range(NJ):
                if P * tau + P - 1 <= BN * j:
                    row.append("full")
                elif P * tau > BN * j + BN - 1:
                    row.append("skip")
                else:
                    row.append(("strip", P * tau - BN * j))
            pat.append(tuple(row))
        return "tril", tuple(pat), 0, None
    # general mask: ship the mixed blocks as multiplicative tiles
    maskT = mask2d.T
    pat = []
    mixes = []
    for tau in range(NT):
        row = []
        for j in range(NJ):
            sub = maskT[tau * P : (tau + 1) * P, j * BN : (j + 1) * BN]
            if sub.all():
                row.append("full")
            elif not sub.any():
                row.append("skip")
            else:
                row.append(("mix", len(mixes)))
                mixes.append(sub.astype(np.float32))
        pat.append(tuple(row))
    mix_arr = (
        np.stack(mixes, axis=1) if mixes else np.zeros((P, 1, BN), np.float32)
    )
    return "general", tuple(pat), len(mixes), np.ascontiguousarray(mix_arr)


def _build(T: int, BN: int, pattern, n_mixed: int, mask_kind: str):
    NT = T // P
    NJ = T // BN
    W = 2 * BN - P  # causal strip width
    MAXO = BN - P
    PJ = min(512, T)  # matmul free-dim chunk for projections
    NPJ = T // PJ

    nc = bacc.Bacc("TRN2", target_bir_lowering=False, debug=False)
    xq = nc.dram_tensor("xq", [T, D], F32, kind="ExternalInput").ap()
    xkv = nc.dram_tensor("xkv", [T, D], F32, kind="ExternalInput").ap()
    wq = nc.dram_tensor("wq", [D, GD], F32, kind="ExternalInput").ap()
    wk = nc.dram_tensor("wk", [D, GD], F32, kind="ExternalInput").ap()
    wv = nc.dram_tensor("wv", [D, GD], F32, kind="ExternalInput").ap()
    wo = nc.dram_tensor("wo", [GD, D], F32, kind="ExternalInput").ap()
    ident_in = nc.dram_tensor("ident", [P, P], F16, kind="ExternalInput").ap()
    ones_in = nc.dram_tensor("ones", [P, NT, 2], F16, kind="ExternalInput").ap()
    hi_in = nc.dram_tensor("hi_init", [P, NT, 2, HD], F16, kind="ExternalInput").ap()
    sel_in = nc.dram_tensor("sel", [NH, 2, P], F16, kind="ExternalInput").ap()
    mstrip = mmix = None
    if mask_kind == "tril":
        mstrip = nc.dram_tensor("mstrip", [P, W], F16, kind="ExternalInput").ap()
    elif mask_kind == "general":
        mmix = nc.dram_tensor(
            "mmix", [P, max(n_mixed, 1), BN], F16, kind="ExternalInput"
        ).ap()
    out = nc.dram_tensor("out", [T, D], F32, kind="ExternalOutput").ap()

    with tile.TileContext(nc) as tc, ExitStack() as ctx:
        pp = ctx.enter_context(tc.tile_pool(name="persist", bufs=1))
        xt = ctx.enter_context(tc.tile_pool(name="xt", bufs=DC + 2))
        p_xrow = ctx.enter_context(tc.tile_pool(name="xrow", bufs=3))
        p_ws = ctx.enter_context(tc.tile_pool(name="wstage", bufs=2))
        p_at = ctx.enter_context(tc.tile_pool(name="atp", bufs=4))
        p_sm = ctx.enter_context(tc.tile_pool(name="small", bufs=2))
        ps_t = ctx.enter_context(tc.tile_pool(name="ps_t", bufs=2, space="PSUM"))
        ps_mm = ctx.enter_context(tc.tile_pool(name="ps_mm", bufs=2, space="PSUM"))
        ps_v = ctx.enter_context(tc.tile_pool(name="ps_v", bufs=2, space="PSUM"))
        ps_o = ctx.enter_context(tc.tile_pool(name="ps_o", bufs=2, space="PSUM"))

        ident = pp.tile([P, P], F16, tag="ident")
        nc.sync.dma_start(ident, ident_in)

        msk = None
        if mask_kind == "tril":
            msk = pp.tile([P, W], F16, tag="mstrip")
            nc.sync.dma_start(msk, mstrip)
        elif mask_kind == "general":
            msk = pp.tile([P, max(n_mixed, 1), BN], F16, tag="mmix")
            nc.sync.dma_start(msk, mmix)

        def load_w16(src_ap, shape, tag):
            stage = p_ws.tile(shape, F32, tag="wstage")
            nc.sync.dma_start(stage, src_ap)
            w16 = pp.tile(shape, F16, tag=tag)
            nc.vector.tensor_copy(w16, stage)
            return w16

        wq_sb = load_w16(wq.rearrange("(c p) g -> p c g", p=P), [P, DC, GD], "wq")
        wk_sb = load_w16(wk.rearrange("(c p) g -> p c g", p=P), [P, DC, GD], "wk")
        wv_sb = load_w16(wv.rearrange("(c p) g -> p c g", p=P), [P, DC, GD], "wv")
        wo_sb = load_w16(
            wo.rearrange("(c p) n -> p c n", p=P), [P, GD // P, D], "wo"
        )

        KT = pp.tile([P, 2, T], F16, tag="KT")  # [d%128, d//128, T']
        QT = pp.tile([P, 2, T], F16, tag="QT")
        # Even heads (OgT partitions 0-63): V in cols 0-63, ones col at 64
        #   -> out partitions 0-63 = O^T, partition 64 = denominator.
        # Odd heads (OgT partitions 64-127): ones col 0, zeros 1-63, V in
        #   cols 64-127 -> partition 0 = denominator (partition_broadcast
        #   only reads absolute partition 0 on HW), partitions 64-127 = O^T.
        Vt_lo = pp.tile([P, NT, 2, HD + 1], F16, tag="Vt_lo")
        Vt_hi = pp.tile([P, NT, 2, P], F16, tag="Vt_hi")
        OgT = pp.tile([P, GD // P, T], F16, tag="OgT")
        nc.sync.dma_start(Vt_lo[:, :, :, HD], ones_in)
        nc.sync.dma_start(Vt_hi[:, :, :, 0:HD], hi_in)
        sel_sb = pp.tile([NH, 2, P], F16, tag="sel")
        nc.sync.dma_start(sel_sb, sel_in)

        def transpose_in(src, dst_tiles):
            for tt in range(NT):
                xrow = p_xrow.tile([P, D], F32, tag="xrow")
                nc.sync.dma_start(xrow, src[ts(tt, P), :])
                xcast = p_xrow.tile([P, D], F16, tag="xcast")
                nc.vector.tensor_copy(xcast, xrow)
                for c in range(DC):
                    pt = ps_t.tile([P, P], F16, tag="tp")
                    nc.tensor.transpose(pt, xcast[:, ts(c, P)], ident)
                    nc.any.tensor_copy(dst_tiles[c][:, ts(tt, P)], pt)

        def project(xT_tiles, w_sb, dstT):
            # dstT[d, t] = sum_D w[D, d] * x[t, D], d-major output
            for dc in range(GD // P):
                for jn in range(NPJ):
                    ps = ps_mm.tile([P, PJ], F32, tag="mm")
                    for c in range(DC):
                        nc.tensor.matmul(
                            ps,
                            lhsT=w_sb[:, c, ts(dc, P)],
                            rhs=xT_tiles[c][:, ts(jn, PJ)],
                            start=(c == 0),
                            stop=(c == DC - 1),
                        )
                    nc.any.tensor_copy(dstT[:, dc, ts(jn, PJ)], ps)

        # ---- K, V from transposed xkv ----
        xkvT = [
            xt.tile([P, T], F16, tag="xt", name=f"xkvT{c}") for c in range(DC)
        ]
        transpose_in(xkv, xkvT)
        project(xkvT, wk_sb, KT)
        for tt in range(NT):
            ps = ps_v.tile([P, GD], F32, tag="vp")
            for c in range(DC):
                nc.tensor.matmul(
                    ps,
                    lhsT=xkvT[c][:, ts(tt, P)],
                    rhs=wv_sb[:, c, :],
                    start=(c == 0),
                    stop=(c == DC - 1),
                )
            for h in range(NH):
                if h % 2 == 0:
                    nc.any.tensor_copy(
                        Vt_lo[:, tt, h // 2, 0:HD], ps[:, ts(h, HD)]
                    )
                else:
                    nc.any.tensor_copy(
                        Vt_hi[:, tt, h // 2, HD:P], ps[:, ts(h, HD)]
                    )

        # ---- Q from transposed xq (reuses the xt slots) ----
        xqT = [xt.tile([P, T], F16, tag="xt", name=f"xqT{c}") for c in range(DC)]
        transpose_in(xq, xqT)
        project(xqT, wq_sb, QT)

        # ---- attention ----
        for j in range(NJ):
            den4 = p_sm.tile([NH, BN], F32, tag="den4")
            for h in range(NH):
                dc, dp = h // 2, (h % 2) * HD
                lo = h % 2 == 0
                po = ps_o.tile([P, BN], F32, tag="po")
                taus = [tau for tau in range(NT) if pattern[tau][j] != "skip"]
                for i, tau in enumerate(taus):
                    blk = pattern[tau][j]
                    # cols [0, off) of this S^T block are fully masked for
                    # every partition row: skip them in matmul/exp/mask.
                    off = blk[1] if blk != "full" and blk[0] == "strip" else 0
                    bw = BN - off
                    ps = ps_mm.tile([P, BN], F32, tag="mm")
                    nc.tensor.matmul(
                        ps[:, off:BN],
                        lhsT=KT[dp : dp + HD, dc, ts(tau, P)],
                        rhs=QT[dp : dp + HD, dc, j * BN + off : (j + 1) * BN],
                        start=True,
                        stop=True,
                    )
                    at = p_at.tile([P, BN], F16, tag="at")
                    nc.scalar.activation(
                        at[:, off:BN], ps[:, off:BN], EXP, scale=0.125
                    )
                    if blk != "full":
                        if blk[0] == "strip":
                            m_ap = msk[:, MAXO : MAXO + bw]
                        else:
                            m_ap = msk[:, blk[1], off:BN]
                        nc.vector.tensor_mul(
                            out=at[:, off:BN], in0=at[:, off:BN], in1=m_ap
                        )
                    vt = (
                        Vt_lo[:, tau, h // 2, :] if lo else Vt_hi[:, tau, h // 2, :]
                    )
                    nc.tensor.matmul(
                        po[0 : HD + 1, off:BN] if lo else po[:, off:BN],
                        lhsT=vt,
                        rhs=at[:, off:BN],
                        start=(i == 0),
                        stop=(i == len(taus) - 1),
                    )
                # stash the denominator row (psum -> sbuf, partition-aligned),
                # hop it into den4[h] (cross-partition via DMA), and copy the
                # unnormalized O^T out so the psum tile frees early.
                dp0 = HD if lo else 0
                stg = p_sm.tile([HD + 1, BN], F32, tag="dstage")
                nc.any.tensor_copy(stg[dp0 : dp0 + 1, :], po[dp0 : dp0 + 1, :])
                nc.sync.dma_start(den4[h : h + 1, :], stg[dp0 : dp0 + 1, :])
                nc.any.tensor_copy(
                    OgT[dp : dp + HD, dc, ts(j, BN)],
                    po[0:HD, :] if lo else po[HD:P, :],
                )
            # one batched reciprocal for all 4 heads of this t-block
            rec4 = p_sm.tile([NH, BN], F16, tag="rec4")
            with nc.allow_low_precision(reason="softmax denom recip in fp16"):
                nc.vector.reciprocal(rec4, den4)
            for c in range(2):
                # bc[p, f] = rec4[head(c, p), f] via a K=4 selection matmul
                bc = ps_mm.tile([P, BN], F32, tag="mm")
                nc.tensor.matmul(
                    bc,
                    lhsT=sel_sb[:, c, :],
                    rhs=rec4,
                    start=True,
                    stop=True,
                )
                nc.vector.tensor_mul(
                    out=OgT[:, c, ts(j, BN)],
                    in0=OgT[:, c, ts(j, BN)],
                    in1=bc,
                )

        # ---- output projection (partial: this core's 256 Wo rows) ----
        for tt in range(NT):
            for n in range(D // PJ):
                ps = ps_mm.tile([P, PJ], F32, tag="mm")
                for c in range(GD // P):
                    nc.tensor.matmul(
                        ps,
                        lhsT=OgT[:, c, ts(tt, P)],
                        rhs=wo_sb[:, c, ts(n, PJ)],
                        start=(c == 0),
                        stop=(c == GD // P - 1),
                    )
                ot = p_sm.tile([P, PJ], F32, tag="ot")
                nc.any.tensor_copy(ot, ps)
                nc.sync.dma_start(out[ts(tt, P), ts(n, PJ)], ot)

    nc.compile()
    return nc


def _strip(BN: int) -> np.ndarray:
    W, MAXO = 2 * BN - P, BN - P
    return np.ascontiguousarray(
        (np.arange(W)[None, :] >= (np.arange(P)[:, None] + MAXO)).astype(np.float32)
    )


def kernel(inputs_q, inputs_kv, mask, Wq, Wk, Wv, Wo):
    global LAST_RESULTS
    inputs_q = np.asarray(inputs_q, np.float32)
    inputs_kv = np.asarray(inputs_kv, np.float32)
    Wq, Wk, Wv, Wo = (np.asarray(w, np.float32) for w in (Wq, Wk, Wv, Wo))
    T = inputs_q.shape[1]
    BN = min(512, T)
    mask2d = np.asarray(mask).reshape(T, T).astype(bool)
    mask_kind, pattern, n_mixed, mix_arr = _classify(mask2d, T, BN)
    key = (T, BN, mask_kind, pattern)
    if key not in _cache:
        _cache[key] = _build(T, BN, pattern, n_mixed, mask_kind)
    nc = _cache[key]

    in_maps = []
    for core in range(N_CORES):
        b, g = divmod(core, N_CORES // B)
        m = {
            "xq": np.ascontiguousarray(inputs_q[b]),
            "xkv": np.ascontiguousarray(inputs_kv[b]),
            "wq": np.ascontiguousarray(Wq[:, g * GD : (g + 1) * GD]),
            "wk": np.ascontiguousarray(Wk[:, g * GD : (g + 1) * GD]),
            "wv": np.ascontiguousarray(Wv[:, g * GD : (g + 1) * GD]),
            "wo": np.ascontiguousarray(Wo[g * GD : (g + 1) * GD, :]),
        }
        m["ident"] = np.ascontiguousarray(np.eye(P, dtype=np.float16))
        sel = np.zeros((NH, 2, P), np.float16)
        for h in range(NH):
            sel[h, h // 2, (h % 2) * HD : (h % 2) * HD + HD] = 1.0
        m["sel"] = sel
        m["ones"] = np.ones((P, T // P, 2), np.float16)
        hi_init = np.zeros((P, T // P, 2, HD), np.float16)
        hi_init[..., 0] = 1.0
        m["hi_init"] = hi_init
        if mask_kind == "tril":
            m["mstrip"] = _strip(BN).astype(np.float16)
        elif mask_kind == "general":
            m["mmix"] = mix_arr.astype(np.float16)
        in_maps.append(m)

    trace = os.environ.get("KERNEL_TRACE", "0") == "1"
    if trace:
        _enable_tracing()
    res = run_bass_kernel_spmd(
        nc, in_maps, core_ids=list(range(N_CORES)), trace=trace
    )
    LAST_RESULTS = res
    parts = [res.results[c]["out"] for c in range(N_CORES)]
    gpb = N_CORES // B
    out = np.stack(
        [sum(parts[b * gpb : (b + 1) * gpb]) for b in range(B)], axis=0
    )
    return out.astype(np.float32)
